# revision 5
# baseline (speedup 1.0000x reference)
"""Trainium2 Bass kernel for blockwise 3D attention (diagonal blocks only).

Reference computation (B=1, C=128, D=H=W=64, NB=8, so db=hb=wb=8, n=512):
  - x is split into an 8x8x8 grid of (8,8,8) spatial blocks; only the 64
    "diagonal" blocks (i, j, k=i) are attended, each independently:
      blk                (C=128, n=512)
      q = Wq blk + bq    (16, n)
      k = Wk blk + bk    (16, n)
      v = Wv blk + bv    (C, n)
      attn = softmax(q^T k, axis=keys)
      ob   = v @ attn^T
      out_blk = gamma * ob + blk
  - everywhere else out = x.

Distribution: the 64 diagonal blocks are sharded 8-per-core across the 8
NeuronCores (block/data parallel); the small 1x1-conv weights are replicated.
The pass-through part of the output (out = x off-diagonal) is assembled on the
host, which is a pure memcpy; all FLOPs for the attended blocks run on device.

Per-core kernel notes:
  - all matmuls run on the float32r PE path (1 cycle/row for moving dims >=
    256 vs 4 cycles/row for plain fp32); every matmul input is produced as an
    explicit f32r tile because the BIR verifier requires f32r operands to come
    from instructions that round to f32r (DMA does not).
  - q/k are computed "packed": lhsT (128,128) has Wq^T (or Wk^T) replicated in
    columns [32r, 32r+16) for r=0..3, zeros elsewhere, so the 4 score matmuls
    (contract dim 16, zero-padded to 32) run row-tiled with
    tile_position=(32*mc, 0) on disjoint 32-row groups of the PE array.
  - scores are computed transposed, ST[m, n] (keys m on partitions), so exp(ST)
    feeds both the AV contraction (contracting m on partitions) and the
    denominator reduction (ones-vector matmul).
  - 1/denom is computed as exp(-ln(denom)) on the scalar engine: Ln and Exp
    share one activation table set, while Reciprocal would force a ~2.7us
    table swap per use (and is disallowed in bass for accuracy).
  - the (1,512) reciprocal row is broadcast to 128 partitions with a K=1
    matmul against a ones row; gamma rides on the v^T copy, gamma*bv on the
    fused final (ob*rbc + gbv) + blk vector op (softmax rows sum to 1, so the
    v-bias contributes exactly bv per output element).
"""

import numpy as np
from contextlib import ExitStack

import concourse.bass as bass
import concourse.bacc as bacc
import concourse.mybir as mybir
from concourse.tile import TileContext
from concourse.bass_utils import run_bass_kernel_spmd

F32 = mybir.dt.float32
F32R = mybir.dt.float32r

NB = 8
C = 128
DB = 8            # block side
N = DB * DB * DB  # 512 voxels per block
NCORES = 8
PPC = NB * NB // NCORES  # 8 diagonal blocks per core

_AR = np.arange(NB)


def build_nc():
    nc = bacc.Bacc()
    blk_d = nc.declare_dram_parameter("blk", [PPC, C, N], F32, isOutput=False)
    wqp_d = nc.declare_dram_parameter("wqp", [C, 128], F32, isOutput=False)
    wkp_d = nc.declare_dram_parameter("wkp", [C, 128], F32, isOutput=False)
    bqp_d = nc.declare_dram_parameter("bqp", [C, 1], F32, isOutput=False)
    bkp_d = nc.declare_dram_parameter("bkp", [C, 1], F32, isOutput=False)
    wvT_d = nc.declare_dram_parameter("wvT", [C, C], F32, isOutput=False)
    gbv_d = nc.declare_dram_parameter("gbv", [C, 1], F32, isOutput=False)
    gam_d = nc.declare_dram_parameter("gam", [C, 1], F32, isOutput=False)
    y_d = nc.declare_dram_parameter("y", [PPC, C, N], F32, isOutput=True)

    Id = mybir.ActivationFunctionType.Identity
    Exp = mybir.ActivationFunctionType.Exp
    Ln = mybir.ActivationFunctionType.Ln
    ADD = mybir.AluOpType.add
    MUL = mybir.AluOpType.mult

    with TileContext(nc) as tc, ExitStack() as ctx:
        singles = ctx.enter_context(tc.tile_pool(name="singles", bufs=1))
        blk_p = ctx.enter_context(tc.tile_pool(name="blk", bufs=3))
        blkr_p = ctx.enter_context(tc.tile_pool(name="blkr", bufs=2))
        qk_p = ctx.enter_context(tc.tile_pool(name="qk", bufs=2))
        vt_p = ctx.enter_context(tc.tile_pool(name="vt", bufs=2))
        e_p = ctx.enter_context(tc.tile_pool(name="e", bufs=2))
        sm_p = ctx.enter_context(tc.tile_pool(name="sm", bufs=2))
        rbc_p = ctx.enter_context(tc.tile_pool(name="rbc", bufs=2))
        y_p = ctx.enter_context(tc.tile_pool(name="y", bufs=3))

        qk_ps = ctx.enter_context(tc.tile_pool(name="qk_ps", bufs=2, space="PSUM"))
        vt_ps_p = ctx.enter_context(tc.tile_pool(name="vt_ps", bufs=1, space="PSUM"))
        st_ps_p = ctx.enter_context(tc.tile_pool(name="st_ps", bufs=1, space="PSUM"))
        den_ps_p = ctx.enter_context(tc.tile_pool(name="den_ps", bufs=1, space="PSUM"))
        rbc_ps_p = ctx.enter_context(tc.tile_pool(name="rbc_ps", bufs=1, space="PSUM"))
        ob_ps_p = ctx.enter_context(tc.tile_pool(name="ob_ps", bufs=1, space="PSUM"))

        # ---- constants: DMA fp32 staging, convert matmul operands to f32r ----
        wqp_f = singles.tile([C, 128], F32)
        wkp_f = singles.tile([C, 128], F32)
        wvT_f = singles.tile([C, C], F32)
        bqp = singles.tile([C, 1], F32)
        bkp = singles.tile([C, 1], F32)
        gbv = singles.tile([C, 1], F32)
        gam = singles.tile([C, 1], F32)
        nc.sync.dma_start(out=wqp_f, in_=wqp_d[:, :])
        nc.sync.dma_start(out=wkp_f, in_=wkp_d[:, :])
        nc.sync.dma_start(out=wvT_f, in_=wvT_d[:, :])
        nc.sync.dma_start(out=bqp, in_=bqp_d[:, :])
        nc.sync.dma_start(out=bkp, in_=bkp_d[:, :])
        nc.sync.dma_start(out=gbv, in_=gbv_d[:, :])
        nc.sync.dma_start(out=gam, in_=gam_d[:, :])
        wqp = singles.tile([C, 128], F32R)
        wkp = singles.tile([C, 128], F32R)
        wvT = singles.tile([C, C], F32R)
        nc.vector.tensor_copy(wqp, wqp_f)
        nc.vector.tensor_copy(wkp, wkp_f)
        nc.vector.tensor_copy(wvT, wvT_f)
        ones_f = singles.tile([C, 1], F32)
        ones1_f = singles.tile([1, C], F32)
        nc.vector.memset(ones_f, 1.0)
        nc.vector.memset(ones1_f, 1.0)
        ones = singles.tile([C, 1], F32R)
        ones1 = singles.tile([1, C], F32R)
        nc.vector.tensor_copy(ones, ones_f)
        nc.vector.tensor_copy(ones1, ones1_f)

        for p in range(PPC):
            blk_f = blk_p.tile([C, N], F32)
            nc.sync.dma_start(out=blk_f, in_=blk_d[p])
            # f32r copy for the PE; gpsimd is otherwise idle
            blk = blkr_p.tile([C, N], F32R)
            nc.gpsimd.tensor_copy(out=blk, in_=blk_f)

            # packed q / k projections -> (128, 512), q_i at partitions 32r+i
            qp_ps = qk_ps.tile([C, N], F32, tag="qkps")
            kp_ps = qk_ps.tile([C, N], F32, tag="qkps")
            nc.tensor.matmul(qp_ps, wqp, blk, start=True, stop=True)
            nc.tensor.matmul(kp_ps, wkp, blk, start=True, stop=True)
            qp = qk_p.tile([C, N], F32R, tag="qksb")
            kp = qk_p.tile([C, N], F32R, tag="qksb")
            nc.scalar.activation(qp, qp_ps, Id, bias=bqp, scale=1.0)
            nc.scalar.activation(kp, kp_ps, Id, bias=bkp, scale=1.0)

            # v^T chunks: vt[:, 128*mc + d] = v[d, 128*mc + m'] ; gamma folded in
            vt_ps = vt_ps_p.tile([C, N], F32)
            for mc in range(4):
                sl = slice(mc * 128, (mc + 1) * 128)
                nc.tensor.matmul(vt_ps[:, sl], blk[:, sl], wvT,
                                 start=True, stop=True)
            vt = vt_p.tile([C, N], F32R)
            nc.vector.tensor_scalar_mul(vt, vt_ps, gam)

            # scores (transposed) + exp, two (128,1024) halves
            e = e_p.tile([C, 2048], F32R)
            for half in range(2):
                st_ps = st_ps_p.tile([C, 1024], F32)
                for j in range(2):
                    mc = 2 * half + j
                    nc.tensor.matmul(
                        st_ps[:, j * N:(j + 1) * N],
                        kp[32 * mc:32 * mc + 32, mc * 128:(mc + 1) * 128],
                        qp[32 * mc:32 * mc + 32, :],
                        start=True, stop=True,
                        tile_position=(32 * mc, 0),
                    )
                nc.scalar.activation(e[:, half * 1024:(half + 1) * 1024], st_ps, Exp)

            # denominator: ones^T E, accumulated over the 4 m-chunks
            den_ps = den_ps_p.tile([1, N], F32)
            for mc in range(4):
                nc.tensor.matmul(den_ps, ones, e[:, mc * N:(mc + 1) * N],
                                 start=(mc == 0), stop=(mc == 3))

            # r' = exp(-ln(den)) = 1/den   (same activation table set as Exp)
            t_sb = sm_p.tile([1, N], F32, tag="t")
            nc.scalar.activation(t_sb, den_ps, Ln)
            r_sb = sm_p.tile([1, N], F32R, tag="r")
            nc.scalar.activation(r_sb, t_sb, Exp, scale=-1.0)

            # broadcast r' to all partitions with a K=1 matmul
            rbc_ps = rbc_ps_p.tile([C, N], F32)
            nc.tensor.matmul(rbc_ps, ones1, r_sb, start=True, stop=True)
            rbc = rbc_p.tile([C, N], F32)
            nc.vector.tensor_copy(rbc, rbc_ps)

            # AV: ob[d, n] = sum_m v[d, m] E[m, n]  (gamma already in vt)
            ob_ps = ob_ps_p.tile([C, N], F32)
            for mc in range(4):
                nc.tensor.matmul(ob_ps, vt[:, mc * 128:(mc + 1) * 128],
                                 e[:, mc * N:(mc + 1) * N],
                                 start=(mc == 0), stop=(mc == 3))

            # y = (ob * rbc + gamma*bv) + blk
            t2 = y_p.tile([C, N], F32, tag="t2")
            nc.vector.tensor_tensor(t2, ob_ps, rbc, op=MUL)
            y_sb = y_p.tile([C, N], F32, tag="y")
            nc.vector.scalar_tensor_tensor(y_sb, t2, gbv, blk_f, op0=ADD, op1=ADD)
            nc.sync.dma_start(out=y_d[p], in_=y_sb)

    return nc


_NC_CACHE = None


def _get_nc():
    global _NC_CACHE
    if _NC_CACHE is None:
        _NC_CACHE = build_nc()
    return _NC_CACHE


def kernel(x, Wq, bq, Wk, bk, Wv, bv, gamma):
    x = np.ascontiguousarray(np.asarray(x, dtype=np.float32))
    Wq = np.asarray(Wq, dtype=np.float32)
    bq = np.asarray(bq, dtype=np.float32)
    Wk = np.asarray(Wk, dtype=np.float32)
    bk = np.asarray(bk, dtype=np.float32)
    Wv = np.asarray(Wv, dtype=np.float32)
    bv = np.asarray(bv, dtype=np.float32)
    g = float(np.asarray(gamma).reshape(-1)[0])

    # ---- extract diagonal blocks: (64, 128, 512) ----
    xv = x.reshape(C, NB, DB, NB, DB, NB, DB)   # (C, Di, dd, Hj, hh, Wk, ww)
    diag = xv[:, _AR, :, :, :, _AR, :]          # (NB=i, C, dd, Hj=j, hh, ww)
    blkall = np.ascontiguousarray(
        diag.transpose(0, 3, 1, 2, 4, 5).reshape(NB * NB, C, N))

    # ---- packed weights ----
    wqp = np.zeros((C, 128), np.float32)
    wkp = np.zeros((C, 128), np.float32)
    bqp = np.zeros((C, 1), np.float32)
    bkp = np.zeros((C, 1), np.float32)
    for r4 in range(4):
        wqp[:, 32 * r4:32 * r4 + 16] = Wq.T
        wkp[:, 32 * r4:32 * r4 + 16] = Wk.T
        bqp[32 * r4:32 * r4 + 16, 0] = bq
        bkp[32 * r4:32 * r4 + 16, 0] = bk
    wvT = np.ascontiguousarray(Wv.T)
    gbv = np.ascontiguousarray((g * bv).reshape(C, 1).astype(np.float32))
    gam = np.full((C, 1), g, np.float32)

    shared = dict(wqp=wqp, wkp=wkp, bqp=bqp, bkp=bkp, wvT=wvT, gbv=gbv, gam=gam)
    in_maps = [
        dict(blk=np.ascontiguousarray(blkall[c * PPC:(c + 1) * PPC]), **shared)
        for c in range(NCORES)
    ]

    nc = _get_nc()
    if not nc.is_finalized():
        nc.finalize()
    res = run_bass_kernel_spmd(nc, in_maps, core_ids=list(range(NCORES)))
    yall = np.concatenate([np.asarray(res.results[c]["y"]) for c in range(NCORES)],
                          axis=0)  # (64, 128, 512)

    # ---- scatter back; off-diagonal blocks pass through x ----
    out = x.copy()
    ov = out.reshape(C, NB, DB, NB, DB, NB, DB)
    ybl = yall.reshape(NB, NB, C, DB, DB, DB)       # (i, j, c, dd, hh, ww)
    ov[:, _AR, :, :, :, _AR, :] = ybl.transpose(0, 2, 3, 1, 4, 5)
    return out.reshape(1, C, NB * DB, NB * DB, NB * DB)


# revision 11
# speedup vs baseline: 1.1531x; 1.1531x over previous
"""Trainium2 Bass kernel for blockwise 3D attention (diagonal blocks only).

Reference computation (B=1, C=128, D=H=W=64, NB=8, so db=hb=wb=8, n=512):
  - x is split into an 8x8x8 grid of (8,8,8) spatial blocks; only the 64
    "diagonal" blocks (i, j, k=i) are attended, each independently:
      blk                (C=128, n=512)
      q = Wq blk + bq    (16, n)
      k = Wk blk + bk    (16, n)
      v = Wv blk + bv    (C, n)
      attn = softmax(q^T k, axis=keys)
      ob   = v @ attn^T
      out_blk = gamma * ob + blk
  - everywhere else out = x.

Distribution: the 64 diagonal blocks are sharded 8-per-core across the 8
NeuronCores (block/data parallel); the small 1x1-conv weights are replicated.
The pass-through part of the output (out = x off-diagonal) is assembled on the
host, which is a pure memcpy; all FLOPs for the attended blocks run on device.

Per-core kernel notes (v2):
  - q/k/score/denominator/AV matmuls use the float32r PE path (1 cycle/row for
    moving dims >= 256 vs 4x for fp32); each f32r operand is produced by an
    engine instruction that rounds to f32r (BIR verifier requirement; DMA
    cannot produce f32r, so the input block is cast fp32->f32r on GpSimd).
  - the v^T projection contracts C with the block chunk as the stationary
    operand and a 128-wide moving dim, where f32r is slow; it runs in bf16
    (host-cast block + Wv^T), which only perturbs the linear v path.
  - q/k are computed "packed": lhsT (128,128) has Wq^T (resp Wk^T) replicated
    in columns [32r, 32r+16), zeros elsewhere, so the 4 score matmuls
    (contract 16, zero-padded to 32) run row-tiled with tile_position=
    (32*mc, 0) concurrently on disjoint 32-row groups of the PE array.
  - scores are computed transposed, ST[m, n] (keys m on partitions), so exp(ST)
    feeds both the AV contraction and the ones-matmul denominator reduction.
  - denominators for a PAIR of blocks land in one (2,512) PSUM tile via
    one-hot-column lhsT (sel_even/sel_odd); 1/den uses the exact DVE
    reciprocal, run on all 128 lanes by DMA-reshaping the pair rows to a
    (128,8) tile through DRAM (Exp stays the only table-based activation,
    so there is exactly one ACT_TABLE_LOAD in the whole kernel).
  - the (2,512) reciprocal rows are broadcast to 128 partitions by a DRAM
    round-trip DMA with a partition-stride-0 source AP (no engine time).
  - gamma rides on the v^T copy, gamma*bv is pre-added to the residual copy
    of the block on the host (softmax rows sum to 1, so v's bias contributes
    exactly bv per output element).
"""

import numpy as np
import ml_dtypes
from contextlib import ExitStack

import concourse.bass as bass
import concourse.bacc as bacc
import concourse.mybir as mybir
from concourse.tile import TileContext
from concourse.bass_utils import run_bass_kernel_spmd

F32 = mybir.dt.float32
F32R = mybir.dt.float32r
BF16 = mybir.dt.bfloat16

NB = 8
C = 128
DB = 8            # block side
N = DB * DB * DB  # 512 voxels per block
NCORES = 8
PPC = NB * NB // NCORES  # 8 diagonal blocks per core

_AR = np.arange(NB)


def build_nc(use_bf16=False):
    nc = bacc.Bacc()
    blk_d = nc.declare_dram_parameter("blk", [PPC, C, N], F32, isOutput=False)
    blk16_d = nc.declare_dram_parameter("blk16", [PPC, C, N], BF16, isOutput=False)
    blkres_d = nc.declare_dram_parameter("blkres", [PPC, C, N], F32, isOutput=False)
    wqp_d = nc.declare_dram_parameter("wqp", [C, 128], F32, isOutput=False)
    wkp_d = nc.declare_dram_parameter("wkp", [C, 128], F32, isOutput=False)
    bqp_d = nc.declare_dram_parameter("bqp", [C, 1], F32, isOutput=False)
    bkp_d = nc.declare_dram_parameter("bkp", [C, 1], F32, isOutput=False)
    wv16_d = nc.declare_dram_parameter("wv16", [C, C], BF16, isOutput=False)
    selc_d = nc.declare_dram_parameter("selc", [C, 4], F32, isOutput=False)
    gam_d = nc.declare_dram_parameter("gam", [C, 1], F32, isOutput=False)
    y_d = nc.declare_dram_parameter("y", [PPC, C, N], F32, isOutput=True)

    Exp = mybir.ActivationFunctionType.Exp
    ADD = mybir.AluOpType.add
    MUL = mybir.AluOpType.mult

    with TileContext(nc) as tc, ExitStack() as ctx:
        singles = ctx.enter_context(tc.tile_pool(name="singles", bufs=1))
        blk_p = ctx.enter_context(tc.tile_pool(name="blk", bufs=3))
        blkr_p = ctx.enter_context(tc.tile_pool(name="blkr", bufs=2))
        qk_p = ctx.enter_context(tc.tile_pool(name="qk", bufs=2))
        vt_p = ctx.enter_context(tc.tile_pool(name="vt", bufs=2))
        e_p = ctx.enter_context(tc.tile_pool(name="e", bufs=2))
        sm_p = ctx.enter_context(tc.tile_pool(name="sm", bufs=2))
        rbc_p = ctx.enter_context(tc.tile_pool(name="rbc", bufs=2))
        y_p = ctx.enter_context(tc.tile_pool(name="y", bufs=3))
        dr_p = ctx.enter_context(tc.tile_pool(name="dr", bufs=2, space="DRAM"))

        qk_ps = ctx.enter_context(tc.tile_pool(name="qk_ps", bufs=2, space="PSUM"))
        vt_ps_p = ctx.enter_context(tc.tile_pool(name="vt_ps", bufs=1, space="PSUM"))
        st_ps_p = ctx.enter_context(tc.tile_pool(name="st_ps", bufs=1, space="PSUM"))
        den_ps_p = ctx.enter_context(tc.tile_pool(name="den_ps", bufs=1, space="PSUM"))
        ob_ps_p = ctx.enter_context(tc.tile_pool(name="ob_ps", bufs=2, space="PSUM"))

        # ---- constants: DMA staging, convert f32r operands once ----
        wqp_f = singles.tile([C, 128], F32)
        wkp_f = singles.tile([C, 128], F32)
        selc_f = singles.tile([C, 4], F32)
        bqp = singles.tile([C, 1], F32)
        bkp = singles.tile([C, 1], F32)
        gam = singles.tile([C, 1], F32)
        wv16 = singles.tile([C, C], BF16)
        nc.sync.dma_start(out=wqp_f, in_=wqp_d[:, :])
        nc.sync.dma_start(out=wkp_f, in_=wkp_d[:, :])
        nc.sync.dma_start(out=selc_f, in_=selc_d[:, :])
        nc.sync.dma_start(out=bqp, in_=bqp_d[:, :])
        nc.sync.dma_start(out=bkp, in_=bkp_d[:, :])
        nc.sync.dma_start(out=gam, in_=gam_d[:, :])
        nc.sync.dma_start(out=wv16, in_=wv16_d[:, :])
        wqp = singles.tile([C, 128], F32R)
        wkp = singles.tile([C, 128], F32R)
        selc = singles.tile([C, 4], F32R)
        nc.vector.tensor_copy(wqp, wqp_f)
        nc.vector.tensor_copy(wkp, wkp_f)
        nc.vector.tensor_copy(selc, selc_f)
        if not use_bf16:
            wv16r = singles.tile([C, C], F32R)
            nc.vector.tensor_copy(wv16r, wv16)

        den2_ps = None
        ob_hold = None
        blkres_hold = None
        for p in range(PPC):
            half = p % 2
            blk_f = blk_p.tile([C, N], F32, tag="blkf")
            blk16 = blk_p.tile([C, N], BF16, tag="blk16")
            blkres = blk_p.tile([C, N], F32, tag="blkres")
            nc.sync.dma_start(out=blk_f, in_=blk_d[p])
            nc.sync.dma_start(out=blk16, in_=blk16_d[p])
            nc.sync.dma_start(out=blkres, in_=blkres_d[p])
            # f32r cast for the PE; GpSimd is otherwise idle
            blk = blkr_p.tile([C, N], F32R)
            nc.gpsimd.tensor_copy(out=blk, in_=blk_f)

            # packed q / k projections -> (128, 512), q_i at partitions 32r+i
            qp_ps = qk_ps.tile([C, N], F32, tag="qkps")
            kp_ps = qk_ps.tile([C, N], F32, tag="qkps")
            nc.tensor.matmul(qp_ps, wqp, blk, start=True, stop=True)
            nc.tensor.matmul(kp_ps, wkp, blk, start=True, stop=True)
            qp = qk_p.tile([C, N], F32R, tag="qksb")
            kp = qk_p.tile([C, N], F32R, tag="qksb")
            Idf = mybir.ActivationFunctionType.Identity
            nc.scalar.activation(qp, qp_ps, Idf, bias=bqp, scale=1.0)
            nc.vector.tensor_scalar_add(kp, kp_ps, bkp)

            # v^T chunks (bf16): vt[:, 128*mc + d] = sign(g) * v[d, 128*mc + m']
            vt_ps = vt_ps_p.tile([C, N], F32)
            for mc in range(4):
                sl = slice(mc * 128, (mc + 1) * 128)
                if use_bf16:
                    nc.tensor.matmul(vt_ps[:, sl], blk16[:, sl], wv16,
                                     start=True, stop=True)
                else:
                    nc.tensor.matmul(vt_ps[:, sl], blk[:, sl], wv16r,
                                     start=True, stop=True)
            vt = vt_p.tile([C, N], F32R)
            nc.vector.tensor_scalar_mul(vt, vt_ps, gam)

            # scores (transposed) + exp, two (128,1024) halves
            e = e_p.tile([C, 2048], F32R)
            for h in range(2):
                st_ps = st_ps_p.tile([C, 1024], F32)
                for j in range(2):
                    mc = 2 * h + j
                    nc.tensor.matmul(
                        st_ps[:, j * N:(j + 1) * N],
                        kp[32 * mc:32 * mc + 32, mc * 128:(mc + 1) * 128],
                        qp[32 * mc:32 * mc + 32, :],
                        start=True, stop=True,
                        tile_position=(32 * mc, 0),
                    )
                nc.scalar.activation(e[:, h * 1024:(h + 1) * 1024], st_ps, Exp)

            # denominator for the pair: one-hot columns put block p's sums
            # into row p%2 of a shared (2,512) PSUM tile
            if half == 0:
                den2_ps = den_ps_p.tile([2, N], F32)
            sel = selc[:, 2 * half:2 * half + 2]
            for mc in range(4):
                nc.tensor.matmul(den2_ps, sel, e[:, mc * N:(mc + 1) * N],
                                 start=(half == 0 and mc == 0),
                                 stop=(half == 1 and mc == 3))

            # AV: ob[d, n] = sum_m v[d, m] E[m, n]
            ob_ps = ob_ps_p.tile([C, N], F32)
            for mc in range(4):
                nc.tensor.matmul(ob_ps, vt[:, mc * 128:(mc + 1) * 128],
                                 e[:, mc * N:(mc + 1) * N],
                                 start=(mc == 0), stop=(mc == 3))

            if half == 0:
                ob_hold = ob_ps
                blkres_hold = blkres
            else:
                # 1/den for the pair: exact DVE reciprocal, run on all 128
                # lanes by round-tripping the (2,512) rows through DRAM as
                # a (128,8) tile
                den_sb = sm_p.tile([2, N], F32, tag="t")
                nc.vector.tensor_copy(den_sb, den2_ps)
                ddr = dr_p.tile([2, N], F32, tag="ddr")
                nc.sync.dma_start(out=ddr, in_=den_sb)
                dflat = sm_p.tile([C, 2 * N // C], F32, tag="df")
                nc.sync.dma_start(
                    out=dflat,
                    in_=ddr[:, :].rearrange("a (p f) -> (a p) f", p=C // 2))
                rfl = sm_p.tile([C, 2 * N // C], F32, tag="r")
                nc.vector.reciprocal(rfl, dflat)
                rdr = dr_p.tile([2, N], F32, tag="rdr")
                nc.sync.dma_start(
                    out=rdr[:, :].rearrange("a (p f) -> (a p) f", p=C // 2),
                    in_=rfl)
                for g in range(2):
                    rbc = rbc_p.tile([C, N], F32, tag="rbc")
                    nc.sync.dma_start(
                        out=rbc,
                        in_=rdr[g:g + 1, :].partition_broadcast(C),
                    )
                    pg = p - 1 + g
                    ob_g = ob_hold if g == 0 else ob_ps
                    blkres_g = blkres_hold if g == 0 else blkres
                    # y = ob * rbc + (blk + gamma*bv)
                    t2 = y_p.tile([C, N], F32, tag="t2")
                    nc.vector.tensor_tensor(t2, ob_g, rbc, op=MUL)
                    y_sb = y_p.tile([C, N], F32, tag="y")
                    nc.vector.tensor_tensor(y_sb, t2, blkres_g, op=ADD)
                    nc.sync.dma_start(out=y_d[pg], in_=y_sb)

    return nc


_NC_CACHE = None


def _get_nc():
    global _NC_CACHE
    if _NC_CACHE is None:
        _NC_CACHE = build_nc()
        if not _NC_CACHE.is_finalized():
            _NC_CACHE.finalize()
    return _NC_CACHE


def kernel(x, Wq, bq, Wk, bk, Wv, bv, gamma):
    x = np.ascontiguousarray(np.asarray(x, dtype=np.float32))
    Wq = np.asarray(Wq, dtype=np.float32)
    bq = np.asarray(bq, dtype=np.float32)
    Wk = np.asarray(Wk, dtype=np.float32)
    bk = np.asarray(bk, dtype=np.float32)
    Wv = np.asarray(Wv, dtype=np.float32)
    bv = np.asarray(bv, dtype=np.float32)
    g = float(np.asarray(gamma).reshape(-1)[0])

    # ---- extract diagonal blocks: (64, 128, 512) ----
    xv = x.reshape(C, NB, DB, NB, DB, NB, DB)   # (C, Di, dd, Hj, hh, Wk, ww)
    diag = xv[:, _AR, :, :, :, _AR, :]          # (NB=i, C, dd, Hj=j, hh, ww)
    blkall = np.ascontiguousarray(
        diag.transpose(0, 3, 1, 2, 4, 5).reshape(NB * NB, C, N))
    blk16all = blkall.astype(ml_dtypes.bfloat16)
    blkresall = blkall + (g * bv).astype(np.float32)[None, :, None]

    # ---- packed weights ----
    wqp = np.zeros((C, 128), np.float32)
    wkp = np.zeros((C, 128), np.float32)
    bqp = np.zeros((C, 1), np.float32)
    bkp = np.zeros((C, 1), np.float32)
    for r4 in range(4):
        wqp[:, 32 * r4:32 * r4 + 16] = Wq.T
        wkp[:, 32 * r4:32 * r4 + 16] = Wk.T
        bqp[32 * r4:32 * r4 + 16, 0] = bq
        bkp[32 * r4:32 * r4 + 16, 0] = bk
    wv16 = np.ascontiguousarray(Wv.T.astype(ml_dtypes.bfloat16))
    gam = np.full((C, 1), np.float32(g), np.float32)

    selc = np.zeros((C, 4), np.float32)
    selc[:, 0] = 1.0  # sel_even: pair-denominator row 0
    selc[:, 3] = 1.0  # sel_odd:  pair-denominator row 1

    shared = dict(wqp=wqp, wkp=wkp, bqp=bqp, bkp=bkp, wv16=wv16,
                  selc=selc, gam=gam)
    in_maps = [
        dict(blk=np.ascontiguousarray(blkall[c * PPC:(c + 1) * PPC]),
             blk16=np.ascontiguousarray(blk16all[c * PPC:(c + 1) * PPC]),
             blkres=np.ascontiguousarray(blkresall[c * PPC:(c + 1) * PPC]),
             **shared)
        for c in range(NCORES)
    ]

    nc = _get_nc()
    res = run_bass_kernel_spmd(nc, in_maps, core_ids=list(range(NCORES)))
    yall = np.concatenate([np.asarray(res.results[c]["y"]) for c in range(NCORES)],
                          axis=0)  # (64, 128, 512)

    # ---- scatter back; off-diagonal blocks pass through x ----
    out = x.copy()
    ov = out.reshape(C, NB, DB, NB, DB, NB, DB)
    ybl = yall.reshape(NB, NB, C, DB, DB, DB)       # (i, j, c, dd, hh, ww)
    ov[:, _AR, :, :, :, _AR, :] = ybl.transpose(0, 2, 3, 1, 4, 5)
    return out.reshape(1, C, NB * DB, NB * DB, NB * DB)
